# revision 1
# baseline (speedup 1.0000x reference)
"""Trainium2 Bass kernel for classical self-attention (B=1, N=4096, D=768, H=12, Hd=64).

Sharding across 8 NeuronCores (zero-collective SPMD):
  24 units = (head h in 0..11, row-half r in {0,1}); core c owns units
  [3c, 3c+2], reordered per core as [U0, U1, U2] with KV head-slots
  (0, 1, 0) so the program is identical on every core:
    U0 = (m2_head, solo_half), U1 = (solo_head, solo_half), U2 = (m2_head, 1-solo_half)
  where m2_head is the head appearing twice among the core's units.

Per core (all matmuls in float32r; out = lhsT.T @ rhs):
  - K^T/V^T/Q^T projections from a row-permuted x^T (key order permuted
    identically for K and V, so softmax/PV are unaffected).
  - scores^T tiles [128 keys, 512 qrows] -> exp on ACT (scale=1/8 folded in)
    -> PV with a ones-column appended to V so the softmax denominator
    accumulates for free in row 64 of the O^T PSUM tile.
  - out_proj partial = O^T.T @ w_out_cols^T, normalized by 1/denominator
    per query row on the way out of PSUM.
Host sums the 24 partial [2048, 768] blocks (12 heads per row-half) and
adds the output bias.
"""
import numpy as np
from functools import partial

H, Hd, N, D = 12, 64, 4096, 768
NC = 8
NKT = N // 128        # 32 key tiles
NQC = 2048 // 512     # 4 q-chunks per unit
KTG = 3               # key tiles per exp group (3 PSUM banks)


def _core_units(c):
    us = [(u // 2, u % 2) for u in range(3 * c, 3 * c + 3)]
    heads = [h for h, _ in us]
    m2 = max(set(heads), key=heads.count)
    solo_head, solo_half = next((h, r) for h, r in us if h != m2)
    return [(m2, solo_half), (solo_head, solo_half), (m2, 1 - solo_half)]


def _prep_core_inputs(c, x, w_qkv, w_out):
    U = _core_units(c)
    solo_half = U[0][1]
    slot_heads = [U[0][0], U[1][0]]

    xT = x.T  # [768, 4096]
    xT_r = np.ascontiguousarray(np.concatenate(
        [xT[:, 2048 * solo_half:2048 * (solo_half + 1)],
         xT[:, 2048 * (1 - solo_half):2048 * (2 - solo_half)]], axis=1))

    wk = np.stack([w_qkv[768 + h * 64: 768 + (h + 1) * 64] for h in slot_heads])
    wv = np.stack([w_qkv[1536 + h * 64: 1536 + (h + 1) * 64] for h in slot_heads])
    wq = np.stack([w_qkv[h * 64:(h + 1) * 64] for h, _ in U])
    # SBUF layouts: w*_l[p, t, m] = w*T[t*128+p, m] so device DMAs are contiguous.
    wk_l = np.ascontiguousarray(wk.reshape(128, 768).T.reshape(6, 128, 128).transpose(1, 0, 2))
    wv_l = np.ascontiguousarray(wv.reshape(128, 768).T.reshape(6, 128, 128).transpose(1, 0, 2))
    wq_l = np.ascontiguousarray(wq.reshape(192, 768).T.reshape(6, 128, 192).transpose(1, 0, 2))
    wo_l = np.ascontiguousarray(
        np.stack([w_out[:, h * 64:(h + 1) * 64].T for h, _ in U]).transpose(1, 0, 2))
    return dict(xT_r=xT_r, wk_l=wk_l, wv_l=wv_l, wq_l=wq_l, wo_l=wo_l,
                ident=np.eye(128, dtype=np.float32),
                ones_col=np.ones((128, 64), np.float32))


def _build_bass():
    import concourse.mybir as mybir
    import concourse.tile as tile
    from concourse import bacc

    f32 = mybir.dt.float32
    f32r = mybir.dt.float32r
    nc = bacc.Bacc(None, target_bir_lowering=False)

    xT_r = nc.dram_tensor("xT_r", [D, N], f32r, kind="ExternalInput")
    wk_l = nc.dram_tensor("wk_l", [128, 6, 128], f32r, kind="ExternalInput")
    wv_l = nc.dram_tensor("wv_l", [128, 6, 128], f32r, kind="ExternalInput")
    wq_l = nc.dram_tensor("wq_l", [128, 6, 192], f32r, kind="ExternalInput")
    wo_l = nc.dram_tensor("wo_l", [64, 3, D], f32r, kind="ExternalInput")
    ident_d = nc.dram_tensor("ident", [128, 128], f32r, kind="ExternalInput")
    ones_d = nc.dram_tensor("ones_col", [128, 64], f32r, kind="ExternalInput")
    out_part = nc.dram_tensor("out_part", [2, 2048, D], f32, kind="ExternalOutput")

    def r(ap):
        return ap

    with tile.TileContext(nc) as tc:
        with (
            tc.tile_pool(name="wpool", bufs=1) as wpool,
            tc.tile_pool(name="big", bufs=1) as big,
            tc.tile_pool(name="expp", bufs=3) as expp,
            tc.tile_pool(name="osb", bufs=2) as osb,
            tc.tile_pool(name="outsb", bufs=3) as outsb,
            tc.tile_pool(name="small", bufs=4) as small,
            tc.tile_pool(name="dram", bufs=2, space="DRAM") as dramp,
        ):
            # ---- load weights ----
            wk_sb = wpool.tile([128, 6, 128], f32r)   # [ktile-part, ktile, 2x64]
            wv_sb = wpool.tile([128, 6, 128], f32r)
            wq_sb = wpool.tile([128, 6, 192], f32r)
            nc.sync.dma_start(out=wk_sb, in_=wk_l[:, :, :])
            nc.sync.dma_start(out=wv_sb, in_=wv_l[:, :, :])
            nc.sync.dma_start(out=wq_sb, in_=wq_l[:, :, :])
            wo_sb = wpool.tile([64, 3, D], f32r)
            nc.sync.dma_start(out=wo_sb, in_=wo_l[:, :, :])
            ident = wpool.tile([128, 128], f32r)
            nc.sync.dma_start(out=ident, in_=ident_d[:, :])

            # ---- projection phase ----
            KT2 = big.tile([128, N], f32r)       # K^T slot-stacked
            QT01 = big.tile([128, 2048], f32r)
            QT2 = big.tile([64, 2048], f32r)
            V_aug = big.tile([128, NKT, 2, 65], f32r)
            # ones column (softmax denominator accumulator) via host constant
            nc.sync.dma_start(out=V_aug[:, :, :, 64],
                              in_=ones_d[:, :].rearrange("p (a b) -> p a b", a=NKT))
            VT2 = big.tile([128, N], f32r)

            # Projection-phase pools close before the attention pools open:
            # PSUM pools reserve banks statically for their lifetime.
            with (
                tc.tile_pool(name="xchunks", bufs=3) as xchunks,
                tc.tile_pool(name="proj_ps", bufs=2, space="PSUM") as proj_ps,
            ):
                for kc in range(8):
                    xc = xchunks.tile([128, 6, 512], f32r)
                    for kt in range(6):
                        nc.sync.dma_start(
                            out=xc[:, kt, :],
                            in_=xT_r[kt * 128:(kt + 1) * 128, kc * 512:(kc + 1) * 512])
                    ps_k = proj_ps.tile([128, 512], f32, tag="ps_k")
                    ps_v = proj_ps.tile([128, 512], f32, tag="ps_v")
                    ps_q = proj_ps.tile([128, 512], f32, tag="ps_q")
                    for kt in range(6):
                        st, sp = (kt == 0), (kt == 5)
                        nc.tensor.matmul(ps_k, r(wk_sb[:, kt, :]), r(xc[:, kt, :]), start=st, stop=sp)
                        nc.tensor.matmul(ps_v, r(wv_sb[:, kt, :]), r(xc[:, kt, :]), start=st, stop=sp)
                        if kc < 4:
                            nc.tensor.matmul(ps_q, r(wq_sb[:, kt, 0:128]), r(xc[:, kt, :]), start=st, stop=sp)
                        else:
                            nc.tensor.matmul(ps_q[0:64], r(wq_sb[:, kt, 128:192]), r(xc[:, kt, :]), start=st, stop=sp)
                    nc.vector.tensor_copy(KT2[:, kc * 512:(kc + 1) * 512], ps_k)
                    nc.vector.tensor_copy(VT2[:, kc * 512:(kc + 1) * 512], ps_v)
                    if kc < 4:
                        nc.vector.tensor_copy(QT01[:, kc * 512:(kc + 1) * 512], ps_q)
                    else:
                        nc.vector.tensor_copy(QT2[:, (kc - 4) * 512:(kc - 3) * 512], ps_q[0:64])

                # ---- V transpose into natural layout (+ones col stays 1.0) ----
                for kt in range(NKT):
                    ps_t = proj_ps.tile([128, 128], f32r, tag="ps_t")
                    nc.tensor.transpose(ps_t, VT2[:, kt * 128:(kt + 1) * 128], ident)
                    nc.vector.tensor_copy(V_aug[:, kt, 0, 0:64], ps_t[:, 0:64])
                    nc.vector.tensor_copy(V_aug[:, kt, 1, 0:64], ps_t[:, 64:128])

            # ---- attention + out_proj per unit ----
            with (
                tc.tile_pool(name="sc_ps", bufs=2, space="PSUM") as sc_ps,
                tc.tile_pool(name="o_ps", bufs=1, space="PSUM") as o_ps,
                tc.tile_pool(name="op_ps", bufs=1, space="PSUM") as op_ps,
            ):
                ktgs = [(g * KTG, min(KTG, NKT - g * KTG)) for g in range((NKT + KTG - 1) // KTG)]
                O_sbs, recips = [], []
                for j, s in enumerate((0, 1, 0)):
                    QT = QT01[0:64] if j == 0 else (QT01[64:128] if j == 1 else QT2)
                    O_sb = osb.tile([65, 2048], f32r, tag=f"O_sb{min(j, 1)}")
                    for qc in range(NQC):
                        O_ps = o_ps.tile([65, 512], f32, tag="O_ps")
                        first = True
                        for g0, glen in ktgs:
                            sc = sc_ps.tile([128, KTG * 512], f32, tag="sc")
                            for i in range(glen):
                                kt = g0 + i
                                nc.tensor.matmul(
                                    sc[:, i * 512:(i + 1) * 512],
                                    KT2[s * 64:(s + 1) * 64, kt * 128:(kt + 1) * 128],
                                    QT[:, qc * 512:(qc + 1) * 512],
                                    start=True, stop=True)
                            ex = expp.tile([128, KTG * 512], f32r, tag="ex")
                            nc.scalar.activation(
                                ex[:, 0:glen * 512], sc[:, 0:glen * 512],
                                mybir.ActivationFunctionType.Exp, scale=0.125)
                            for i in range(glen):
                                kt = g0 + i
                                nc.tensor.matmul(
                                    O_ps, V_aug[:, kt, s, :], ex[:, i * 512:(i + 1) * 512],
                                    start=first, stop=(kt == NKT - 1))
                                first = False
                        nc.vector.tensor_copy(O_sb[:, qc * 512:(qc + 1) * 512], O_ps)

                    sums_d = dramp.tile([1, 2048], f32, tag="sums_d")
                    nc.sync.dma_start(out=sums_d, in_=O_sb[64:65, :].bitcast(f32))
                    sums_t = small.tile([128, 16], f32, tag=f"sums{min(j, 1)}")
                    nc.sync.dma_start(
                        out=sums_t,
                        in_=sums_d.rearrange("o (t p) -> (o p) t", p=128))
                    recip = small.tile([128, 16], f32, tag=f"recip{min(j, 1)}")
                    nc.vector.reciprocal(recip, sums_t)
                    O_sbs.append(O_sb)
                    recips.append(recip)

                    if j == 0:
                        continue
                    if j == 1:
                        # merged out_proj for U0+U1 (same query rows)
                        pairs = [(O_sbs[0], recips[0], 0), (O_sbs[1], recips[1], 1)]
                        slot = 0
                    else:
                        pairs = [(O_sbs[2], recips[2], 2)]
                        slot = 1
                    for rt in range(16):
                        ob = outsb.tile([128, 768], f32, tag="ob")
                        for pi, (O_u, rc_u, ju) in enumerate(pairs):
                            lhsT = O_u[0:64, rt * 128:(rt + 1) * 128]
                            po1 = op_ps.tile([128, 512], f32, tag="po")
                            nc.tensor.matmul(po1, lhsT, wo_sb[:, ju, 0:512], start=True, stop=True)
                            po2 = op_ps.tile([128, 512], f32, tag="po")
                            nc.tensor.matmul(po2[:, 0:256], lhsT, wo_sb[:, ju, 512:768], start=True, stop=True)
                            if pi == 0:
                                nc.vector.tensor_scalar_mul(ob[:, 0:512], po1, rc_u[:, rt:rt + 1])
                                nc.vector.tensor_scalar_mul(ob[:, 512:768], po2[:, 0:256], rc_u[:, rt:rt + 1])
                            else:
                                tmp = outsb.tile([128, 768], f32, tag="tmp")
                                nc.vector.tensor_scalar_mul(tmp[:, 0:512], po1, rc_u[:, rt:rt + 1])
                                nc.vector.tensor_scalar_mul(tmp[:, 512:768], po2[:, 0:256], rc_u[:, rt:rt + 1])
                                nc.vector.tensor_add(ob, ob, tmp)
                        nc.sync.dma_start(out=out_part[slot, rt * 128:(rt + 1) * 128, :], in_=ob)
    nc.compile()
    return nc


_NC_CACHE = None
_EXEC_CACHE = None


def _install_neff_disk_cache():
    """Persist compiled bass NEFFs across processes (walrus takes minutes)."""
    import hashlib
    import os

    try:
        import libneuronxla
    except ImportError:
        return
    if getattr(libneuronxla, "_bass_neff_disk_cache", False):
        return
    inner = libneuronxla.neuronx_cc
    cachedir = os.path.expanduser("~/.bass_neff_cache")
    os.makedirs(cachedir, exist_ok=True)

    def cached_cc(code, code_format, platform_version, file_prefix):
        if b"bass_exec" not in code:
            return inner(code, code_format, platform_version, file_prefix)
        key = hashlib.sha256(
            repr((code_format, platform_version)).encode() + code).hexdigest()
        path = os.path.join(cachedir, key + ".neff_cc")
        if os.path.exists(path):
            with open(path, "rb") as f:
                return 0, f.read()
        ret = inner(code, code_format, platform_version, file_prefix)
        status, data = ret
        if status == 0:
            tmp = path + ".tmp"
            with open(tmp, "wb") as f:
                f.write(data)
            os.replace(tmp, path)
        return ret

    libneuronxla.neuronx_cc = cached_cc
    libneuronxla._bass_neff_disk_cache = True


def _get_executor():
    """Build (once) a cached sharded jit wrapping the bass NEFF.

    Mirrors concourse.bass2jax.run_bass_via_pjrt but hoists the jitted
    executable into a module-level cache so repeat kernel() calls skip
    retracing/recompiling.
    """
    global _NC_CACHE, _EXEC_CACHE
    if _EXEC_CACHE is not None:
        return _EXEC_CACHE

    import jax
    import concourse.mybir as mybir
    from jax.sharding import Mesh, PartitionSpec
    from jax.experimental.shard_map import shard_map
    from concourse.bass2jax import (
        _bass_exec_p, install_neuronx_cc_hook, partition_id_tensor)

    install_neuronx_cc_hook()
    _install_neff_disk_cache()

    if _NC_CACHE is None:
        _NC_CACHE = _build_bass()
    nc = _NC_CACHE
    partition_name = nc.partition_id_tensor.name if nc.partition_id_tensor else None

    in_names, out_names, out_avals, zero_shapes = [], [], [], []
    for alloc in nc.m.functions[0].allocations:
        if not isinstance(alloc, mybir.MemoryLocationSet):
            continue
        name = alloc.memorylocations[0].name
        if alloc.kind == "ExternalInput":
            if name != partition_name:
                in_names.append(name)
        elif alloc.kind == "ExternalOutput":
            shape = tuple(alloc.tensor_shape)
            dtype = mybir.dt.np(alloc.dtype)
            out_names.append(name)
            out_avals.append(jax.core.ShapedArray(shape, dtype))
            zero_shapes.append((shape, dtype))
    n_params = len(in_names)
    all_names = in_names + out_names
    if partition_name is not None:
        all_names = all_names + [partition_name]

    def _body(*args):
        operands = list(args)
        if partition_name is not None:
            operands.append(partition_id_tensor())
        outs = _bass_exec_p.bind(
            *operands,
            out_avals=tuple(out_avals),
            in_names=tuple(all_names),
            out_names=tuple(out_names),
            lowering_input_output_aliases=(),
            sim_require_finite=True,
            sim_require_nnan=True,
            nc=nc,
        )
        return tuple(outs)

    devices = jax.devices()[:NC]
    mesh = Mesh(np.asarray(devices), ("core",))
    donate = tuple(range(n_params, n_params + len(out_names)))
    sharded = jax.jit(
        shard_map(
            _body, mesh=mesh,
            in_specs=(PartitionSpec("core"),) * (n_params + len(out_names)),
            out_specs=(PartitionSpec("core"),) * len(out_names),
            check_rep=False,
        ),
        donate_argnums=donate, keep_unused=True,
    )

    # Donated output buffers built on-device (no bass_exec -> stock compile
    # path): avoids shipping ~150MB of zeros over the axon tunnel per call.
    import jax.numpy as jnp
    from jax.sharding import NamedSharding

    zero_shardings = tuple(NamedSharding(mesh, PartitionSpec("core"))
                           for _ in zero_shapes)

    @partial(jax.jit, out_shardings=zero_shardings)
    def _make_zeros():
        return tuple(jnp.zeros((NC * s[0], *s[1:]), d) for s, d in zero_shapes)

    _EXEC_CACHE = (sharded, in_names, out_names, out_avals, _make_zeros)
    return _EXEC_CACHE


def kernel(x, w_qkv, w_out, b_out):
    x = np.ascontiguousarray(np.asarray(x, dtype=np.float32))
    w_qkv = np.ascontiguousarray(np.asarray(w_qkv, dtype=np.float32))
    w_out = np.ascontiguousarray(np.asarray(w_out, dtype=np.float32))
    b_out = np.ascontiguousarray(np.asarray(b_out, dtype=np.float32))
    x2 = x[0]

    sharded, in_names, out_names, out_avals, make_zeros = _get_executor()

    in_maps = [_prep_core_inputs(c, x2, w_qkv, w_out) for c in range(NC)]
    concat_in = [
        np.concatenate([in_maps[c][name] for c in range(NC)], axis=0)
        for name in in_names
    ]
    out_arrs = sharded(*concat_in, *make_zeros())

    out = np.zeros((N, D), np.float32)
    parts = np.asarray(out_arrs[out_names.index("out_part")]).reshape(NC, 2, 2048, D)
    for c in range(NC):
        U = _core_units(c)
        out[U[0][1] * 2048:(U[0][1] + 1) * 2048] += parts[c, 0]
        out[U[2][1] * 2048:(U[2][1] + 1) * 2048] += parts[c, 1]
    out += b_out
    return out[None].astype(np.float32)



# revision 2
# speedup vs baseline: 17.0094x; 17.0094x over previous
"""Trainium2 Bass kernel for classical self-attention (B=1, N=4096, D=768, H=12, Hd=64).

Sharding across 8 NeuronCores: query rows. Core c owns query rows
[512c, 512c+512) and produces those output rows completely (all 12
heads + out_proj + bias), so there is no output reduction and every
core runs an identical program.

Wall-clock on the axon tunnel is transfer-bound (~50-75 MB/s each way,
~70ms per jit dispatch), so the per-call I/O is minimized:
  - x ships once as bf16 [4096, 768] (6.3MB), sharded by rows.
  - a small on-device prep jit all-gathers x, transposes it, and
    builds the donated output buffer (no extra host round trips).
  - weight layouts are host-prepped once and cached on device keyed by
    checksum; repeat calls ship nothing but x.
  - the bass kernel computes in bf16 (f32 PSUM) and writes bf16 output
    (6.3MB back), which test tolerance (2e-2) easily absorbs.

Per core (out = lhsT.T @ rhs convention):
  - K^T [768, 4096] and Q^T [768, 512] tiled projections from xT; V is
    produced directly in natural [key, dim] layout by using the x chunk
    as the stationary operand, so no PE transposes are needed.
  - per head: scores^T tiles [128 keys, 512 q] -> exp (scale=1/8) in
    groups of 3 key tiles -> PV with a ones column appended to V so the
    softmax denominator accumulates for free in row 64 of O^T.
  - out_proj accumulates per head with per-head 1/denom scaling fused
    into the PSUM->SBUF copy; the output bias rides head 0's matmul as
    a 65th contraction row against the denominator (d*b trick), so the
    final result needs no separate bias pass.
"""
import numpy as np
import zlib
from functools import partial

H, Hd, N, D = 12, 64, 4096, 768
NC = 8
NQ = N // NC          # 512 own query rows per core
NKT = N // 128        # 32 key tiles
KTG = 3               # key tiles per exp group


def _build_bass():
    import concourse.mybir as mybir
    import concourse.tile as tile
    from concourse import bacc

    f32 = mybir.dt.float32
    bf16 = mybir.dt.bfloat16
    nc = bacc.Bacc(None, target_bir_lowering=False)

    xT = nc.dram_tensor("xT", [D, N], bf16, kind="ExternalInput")
    xqT = nc.dram_tensor("xqT", [D, NQ], bf16, kind="ExternalInput")
    wk_l = nc.dram_tensor("wk_l", [128, 6, D], bf16, kind="ExternalInput")
    wv_l = nc.dram_tensor("wv_l", [128, 6, D], bf16, kind="ExternalInput")
    wq_l = nc.dram_tensor("wq_l", [128, 6, D], bf16, kind="ExternalInput")
    wo_l = nc.dram_tensor("wo_l", [65, H, D], bf16, kind="ExternalInput")
    ones_l = nc.dram_tensor("ones_l", [128, NKT * H], bf16, kind="ExternalInput")
    out = nc.dram_tensor("out", [NQ, D], bf16, kind="ExternalOutput")

    with tile.TileContext(nc) as tc:
        with (
            tc.tile_pool(name="wpool", bufs=1) as wpool,
            tc.tile_pool(name="big", bufs=1) as big,
            tc.tile_pool(name="expp", bufs=2) as expp,
            tc.tile_pool(name="small", bufs=2) as small,
            tc.tile_pool(name="outp", bufs=2) as outp,
            tc.tile_pool(name="dram", bufs=2, space="DRAM") as dramp,
        ):
            # ---- load weights ----
            wk_sb = wpool.tile([128, 6, D], bf16)
            wv_sb = wpool.tile([128, 6, D], bf16)
            wq_sb = wpool.tile([128, 6, D], bf16)
            wo_sb = wpool.tile([65, H, D], bf16)
            nc.sync.dma_start(out=wk_sb, in_=wk_l[:, :, :])
            nc.sync.dma_start(out=wv_sb, in_=wv_l[:, :, :])
            nc.sync.dma_start(out=wq_sb, in_=wq_l[:, :, :])
            nc.sync.dma_start(out=wo_sb, in_=wo_l[:, :, :])

            # ---- persistent activation tiles ----
            KT = big.tile([128, 6, N], bf16)        # K^T tiled [p, dt, key]
            QT = big.tile([128, 6, NQ], bf16)       # Q^T tiled [p, dt, q]
            V_aug = big.tile([128, NKT, H, 65], bf16)  # V natural + ones col
            O_all = big.tile([65, H, NQ], bf16)     # O^T per head + denom row
            nc.sync.dma_start(out=V_aug[:, :, :, 64],
                              in_=ones_l[:, :].rearrange("p (a b) -> p a b", a=NKT))

            # ---- projection phase ----
            with (
                tc.tile_pool(name="xch", bufs=3) as xch,
                tc.tile_pool(name="proj_ps", bufs=2, space="PSUM") as proj_ps,
            ):
                # Q^T for own 512 rows
                xq_sb = xch.tile([128, 6, NQ], bf16, tag="xc")
                for it in range(6):
                    nc.sync.dma_start(out=xq_sb[:, it, :],
                                      in_=xqT[it * 128:(it + 1) * 128, :])
                for dt in range(6):
                    ps_q = proj_ps.tile([128, NQ], f32, tag="ps")
                    for it in range(6):
                        nc.tensor.matmul(ps_q, wq_sb[:, it, dt * 128:(dt + 1) * 128],
                                         xq_sb[:, it, :], start=(it == 0), stop=(it == 5))
                    nc.vector.tensor_copy(QT[:, dt, :], ps_q)

                # K^T (per 512-key chunk) and V natural (per 128-key tile)
                for kc in range(8):
                    xc = xch.tile([128, 6, 512], bf16, tag="xc")
                    for it in range(6):
                        nc.sync.dma_start(
                            out=xc[:, it, :],
                            in_=xT[it * 128:(it + 1) * 128, kc * 512:(kc + 1) * 512])
                    for dt in range(6):
                        ps_k = proj_ps.tile([128, 512], f32, tag="ps")
                        for it in range(6):
                            nc.tensor.matmul(ps_k, wk_sb[:, it, dt * 128:(dt + 1) * 128],
                                             xc[:, it, :], start=(it == 0), stop=(it == 5))
                        nc.vector.tensor_copy(KT[:, dt, kc * 512:(kc + 1) * 512], ps_k)
                    for sub in range(4):
                        kt = kc * 4 + sub
                        ps_v1 = proj_ps.tile([128, 512], f32, tag="psv1")
                        ps_v2 = proj_ps.tile([128, 256], f32, tag="psv2")
                        for it in range(6):
                            st, sp = (it == 0), (it == 5)
                            nc.tensor.matmul(ps_v1, xc[:, it, sub * 128:(sub + 1) * 128],
                                             wv_sb[:, it, 0:512], start=st, stop=sp)
                            nc.tensor.matmul(ps_v2, xc[:, it, sub * 128:(sub + 1) * 128],
                                             wv_sb[:, it, 512:768], start=st, stop=sp)
                        for h in range(8):
                            nc.vector.tensor_copy(V_aug[:, kt, h, 0:64],
                                                  ps_v1[:, h * 64:(h + 1) * 64])
                        for h in range(8, 12):
                            nc.vector.tensor_copy(V_aug[:, kt, h, 0:64],
                                                  ps_v2[:, (h - 8) * 64:(h - 7) * 64])

            # ---- attention per head ----
            ktgs = [(g * KTG, min(KTG, NKT - g * KTG))
                    for g in range((NKT + KTG - 1) // KTG)]
            with (
                tc.tile_pool(name="sc_ps", bufs=2, space="PSUM") as sc_ps,
                tc.tile_pool(name="o_ps", bufs=2, space="PSUM") as o_ps,
            ):
                for h in range(H):
                    dt, dr = h // 2, (h % 2) * 64
                    O_ps = o_ps.tile([65, NQ], f32, tag="O")
                    first = True
                    for g0, glen in ktgs:
                        sc = sc_ps.tile([128, KTG * 512], f32, tag="sc")
                        for i in range(glen):
                            kt = g0 + i
                            nc.tensor.matmul(
                                sc[:, i * 512:(i + 1) * 512],
                                KT[dr:dr + 64, dt, kt * 128:(kt + 1) * 128],
                                QT[dr:dr + 64, dt, :],
                                start=True, stop=True)
                        ex = expp.tile([128, KTG * 512], bf16, tag="ex")
                        nc.scalar.activation(
                            ex[:, 0:glen * 512], sc[:, 0:glen * 512],
                            mybir.ActivationFunctionType.Exp, scale=0.125)
                        for i in range(glen):
                            kt = g0 + i
                            nc.tensor.matmul(O_ps, V_aug[:, kt, h, :],
                                             ex[:, i * 512:(i + 1) * 512],
                                             start=first, stop=(kt == NKT - 1))
                            first = False
                    nc.vector.tensor_copy(O_all[0:65, h, :], O_ps)

            # ---- denominators -> per-token reciprocals [128, H*4] ----
            scr = dramp.tile([1, H * NQ], bf16, tag="scr")
            nc.sync.dma_start(out=scr, in_=O_all[64:65, :, :])
            dsb = small.tile([128, H * 4], bf16, tag="dsb")
            nc.sync.dma_start(
                out=dsb,
                in_=scr.rearrange("a (h c p) -> (a p) (h c)", h=H, p=128))
            dfl = small.tile([128, H * 4], f32, tag="dfl")
            nc.vector.tensor_copy(dfl, dsb)
            recip = small.tile([128, H * 4], f32, tag="recip")
            nc.vector.reciprocal(recip, dfl)

            # ---- out_proj per 128-token chunk ----
            with tc.tile_pool(name="op_ps", bufs=2, space="PSUM") as op_ps:
                for tci in range(4):
                    ob = outp.tile([128, D], f32, tag="ob")
                    tmp = outp.tile([128, D], f32, tag="tmp")
                    for h in range(H):
                        hi = 65 if h == 0 else 64
                        lhsT = O_all[0:hi, h, tci * 128:(tci + 1) * 128]
                        po1 = op_ps.tile([128, 512], f32, tag="po1")
                        po2 = op_ps.tile([128, 256], f32, tag="po2")
                        nc.tensor.matmul(po1, lhsT, wo_sb[0:hi, h, 0:512],
                                         start=True, stop=True)
                        nc.tensor.matmul(po2, lhsT, wo_sb[0:hi, h, 512:768],
                                         start=True, stop=True)
                        r = recip[:, h * 4 + tci:h * 4 + tci + 1]
                        dst = ob if h == 0 else tmp
                        nc.vector.tensor_scalar_mul(dst[:, 0:512], po1, r)
                        nc.vector.tensor_scalar_mul(dst[:, 512:768], po2, r)
                        if h > 0:
                            nc.vector.tensor_add(ob, ob, tmp)
                    osb = outp.tile([128, D], bf16, tag="osb")
                    nc.vector.tensor_copy(osb, ob)
                    nc.sync.dma_start(out=out[tci * 128:(tci + 1) * 128, :], in_=osb)
    nc.compile()
    return nc


_NC_CACHE = None
_EXEC_CACHE = None
_PREP_CACHE = None
_WEIGHT_CACHE = {}


def _install_neff_disk_cache():
    """Persist compiled NEFFs across processes (walrus takes minutes)."""
    import hashlib
    import os

    try:
        import libneuronxla
    except ImportError:
        return
    if getattr(libneuronxla, "_bass_neff_disk_cache", False):
        return
    inner = libneuronxla.neuronx_cc
    cachedir = os.path.expanduser("~/.bass_neff_cache")
    os.makedirs(cachedir, exist_ok=True)

    def cached_cc(code, code_format, platform_version, file_prefix):
        key = hashlib.sha256(
            repr((code_format, platform_version)).encode() + code).hexdigest()
        path = os.path.join(cachedir, key + ".neff_cc")
        if os.path.exists(path):
            with open(path, "rb") as f:
                return 0, f.read()
        ret = inner(code, code_format, platform_version, file_prefix)
        status, data = ret
        if status == 0:
            tmp = path + ".tmp"
            with open(tmp, "wb") as f:
                f.write(data)
            os.replace(tmp, path)
        return ret

    libneuronxla.neuronx_cc = cached_cc
    libneuronxla._bass_neff_disk_cache = True


def _mesh():
    import jax
    from jax.sharding import Mesh
    return Mesh(np.asarray(jax.devices()[:NC]), ("core",))


def _get_executor():
    """Build (once) a cached sharded jit wrapping the bass NEFF."""
    global _NC_CACHE, _EXEC_CACHE
    if _EXEC_CACHE is not None:
        return _EXEC_CACHE

    import jax
    import concourse.mybir as mybir
    from jax.sharding import PartitionSpec
    from jax.experimental.shard_map import shard_map
    from concourse.bass2jax import (
        _bass_exec_p, install_neuronx_cc_hook, partition_id_tensor)

    install_neuronx_cc_hook()
    _install_neff_disk_cache()

    if _NC_CACHE is None:
        _NC_CACHE = _build_bass()
    nc = _NC_CACHE
    partition_name = nc.partition_id_tensor.name if nc.partition_id_tensor else None

    in_names, out_names, out_avals = [], [], []
    for alloc in nc.m.functions[0].allocations:
        if not isinstance(alloc, mybir.MemoryLocationSet):
            continue
        name = alloc.memorylocations[0].name
        if alloc.kind == "ExternalInput":
            if name != partition_name:
                in_names.append(name)
        elif alloc.kind == "ExternalOutput":
            shape = tuple(alloc.tensor_shape)
            dtype = mybir.dt.np(alloc.dtype)
            out_names.append(name)
            out_avals.append(jax.core.ShapedArray(shape, dtype))
    n_params = len(in_names)
    all_names = in_names + out_names
    if partition_name is not None:
        all_names = all_names + [partition_name]

    def _body(*args):
        operands = list(args)
        if partition_name is not None:
            operands.append(partition_id_tensor())
        outs = _bass_exec_p.bind(
            *operands,
            out_avals=tuple(out_avals),
            in_names=tuple(all_names),
            out_names=tuple(out_names),
            lowering_input_output_aliases=(),
            sim_require_finite=True,
            sim_require_nnan=True,
            nc=nc,
        )
        return tuple(outs)

    mesh = _mesh()
    donate = tuple(range(n_params, n_params + len(out_names)))
    sharded = jax.jit(
        shard_map(
            _body, mesh=mesh,
            in_specs=(PartitionSpec("core"),) * (n_params + len(out_names)),
            out_specs=(PartitionSpec("core"),) * len(out_names),
            check_rep=False,
        ),
        donate_argnums=donate, keep_unused=True,
    )

    _EXEC_CACHE = (sharded, in_names, out_names)
    return _EXEC_CACHE


def _get_prep():
    """Jit that uploads x (bf16, sharded by rows), all-gathers + transposes
    it on device, and builds the donated output buffer."""
    global _PREP_CACHE
    if _PREP_CACHE is not None:
        return _PREP_CACHE
    import jax
    import jax.numpy as jnp
    from jax.sharding import PartitionSpec as P
    from jax.experimental.shard_map import shard_map

    mesh = _mesh()

    def body(xs):
        xg = jax.lax.all_gather(xs, "core", axis=0, tiled=True)  # [4096, 768]
        return xg.T, xs.T, jnp.zeros((NQ, D), jnp.bfloat16)

    _PREP_CACHE = jax.jit(shard_map(
        body, mesh=mesh, in_specs=(P("core"),),
        out_specs=(P("core"),) * 3, check_rep=False))
    return _PREP_CACHE


def _bf16(a):
    import ml_dtypes
    return np.ascontiguousarray(a.astype(ml_dtypes.bfloat16))


def _get_weights_dev(w_qkv, w_out, b_out):
    """Host-prep weight layouts, upload once, cache device arrays."""
    key = (zlib.crc32(w_qkv.tobytes()), zlib.crc32(w_out.tobytes()),
           zlib.crc32(b_out.tobytes()))
    hit = _WEIGHT_CACHE.get(key)
    if hit is not None:
        return hit

    import jax
    from jax.sharding import NamedSharding, PartitionSpec as P

    def tile_w(w):  # [768 out, 768 in] -> [128, 6, 768]: [p, it, o] = w[o, it*128+p]
        return np.ascontiguousarray(w.T.reshape(6, 128, D).transpose(1, 0, 2))

    wq_ = tile_w(w_qkv[0:D])
    wk_ = tile_w(w_qkv[D:2 * D])
    wv_ = tile_w(w_qkv[2 * D:3 * D])
    wo_ = np.zeros((65, H, D), np.float32)
    wo_[0:64] = w_out.T.reshape(H, Hd, D).transpose(1, 0, 2)
    wo_[64, 0, :] = b_out
    ones_ = np.ones((128, NKT * H), np.float32)

    sh = NamedSharding(_mesh(), P("core"))
    dev = {}
    for name, arr in (("wk_l", wk_), ("wv_l", wv_), ("wq_l", wq_),
                      ("wo_l", wo_), ("ones_l", ones_)):
        rep = _bf16(np.broadcast_to(arr[None], (NC,) + arr.shape).reshape(
            (NC * arr.shape[0],) + arr.shape[1:]))
        dev[name] = jax.device_put(rep, sh)
    jax.block_until_ready(list(dev.values()))
    _WEIGHT_CACHE.clear()  # hold at most one weight set on device
    _WEIGHT_CACHE[key] = dev
    return dev


def kernel(x, w_qkv, w_out, b_out):
    x = np.asarray(x, dtype=np.float32)
    w_qkv = np.ascontiguousarray(np.asarray(w_qkv, dtype=np.float32))
    w_out = np.ascontiguousarray(np.asarray(w_out, dtype=np.float32))
    b_out = np.ascontiguousarray(np.asarray(b_out, dtype=np.float32))

    sharded, in_names, out_names = _get_executor()
    wdev = _get_weights_dev(w_qkv, w_out, b_out)
    prep = _get_prep()

    xb = _bf16(x[0])
    xT_g, xqT_g, z_g = prep(xb)
    args = dict(wdev)
    args["xT"] = xT_g
    args["xqT"] = xqT_g
    out_arrs = sharded(*[args[n] for n in in_names], z_g)
    res = np.asarray(out_arrs[out_names.index("out")]).astype(np.float32)
    return res[None]
